# revision 16
# baseline (speedup 1.0000x reference)
"""Differential multi-head attention Trainium2 Bass kernel, v6.

Problem: B=4, N=1024, D=512, H=8 heads, DH=64. LAM=0.5.
  q = (x@Wq+bq)  -> [B,H,N,2*DH], halves q1,q2 (same for k)
  a_i = softmax(q_i@k_i^T / sqrt(DH)); attn = a1 - LAM*a2; out = attn@v

Sharding: core c handles batch c//2, heads (c%2)*4..+3.

Software-pipelined steady-state body (For_i reps): head h3's PV chains
and finishes WRAP to the next iteration (consumed at the top of the
body, while h0's scores run), so the per-rep marginal time has no
lead-in/tail bubble. prologue does the first h0 projection + consts;
epilogue runs the final h3 chains + output DMA. es pool bufs=16 aligns
slot reuse so e(3,kt) [slots 8-15] survive until e(1,0) of the next
rep (step 9).

Per-head engine balance (~16us target):
  - ACT: exp [128,1024] PSUM->SBUF bf16, 2/(h,kt) minus OFF_HALVES
    offloaded to DVE as cubic poly in w-form (w=e-1, coeffs absorb the
    1/8 scale); chains corrected with vaug-colsum (corr).
  - PE: projections (one weight load per (h,dc), two qc matmuls),
    tile-packed score pairs, PV chains with 66-wide aug (col 64 = 1 ->
    r1; col 65 = -1/LAM -> 1/u[65] = -LAM/r2 folds -LAM into the
    reciprocal), fp16 transpose-mode finishes.
  - DVE: q/k/v bias adds, u copies (+corr), poly tiles, fp16 2x-mode
    finish muls, fp16 ostage/output.
"""
import sys

sys.path.insert(0, "/opt/trn_rl_repo")

from contextlib import ExitStack

import numpy as np

import concourse.bass as bass
import concourse.mybir as mybir
import concourse.tile as tile
from concourse import bacc, bass_utils
from concourse.masks import make_identity

F32 = mybir.dt.float32
F16 = mybir.dt.float16
BF16 = mybir.dt.bfloat16

B, N, D, H = 4, 1024, 512, 8
DH = 64
HPC = 4
LAM = 0.5
SCALE = 0.125
NCORES = 8
CQ = 512
CV = 256
P = 128
NT = 8
DC = 4
QW = 512
AUG = DH + 2  # [v | 1 | -1/LAM]

# exp(s)-1 ~ s*(C1 + s*(C2 + s*C3)) for s = SCALE*z, z the raw score.
# Coefficients absorb SCALE so the poly runs on raw scores.
PC1 = 1.03696098 * SCALE
PC2 = 0.54989007 * SCALE * SCALE
PC3 = 0.14098758 * SCALE * SCALE * SCALE

# (kt, half) pairs whose exp is computed on DVE as poly (w-form).
OFF_HALVES = frozenset({(3, 0), (3, 1)})
OFF_KTS = sorted({kt for kt, _ in OFF_HALVES})

PUMP_NS = 1800


def build_nc(reps=1):
    nc = bacc.Bacc("TRN2", target_bir_lowering=False, debug=False,
                   num_devices=NCORES)
    d = {
        "xt": nc.dram_tensor("xt", [D, N], BF16, kind="ExternalInput"),
        "wq": nc.dram_tensor("wq", [D, CQ], BF16, kind="ExternalInput"),
        "wk": nc.dram_tensor("wk", [D, CQ], BF16, kind="ExternalInput"),
        "wv": nc.dram_tensor("wv", [D, CV], BF16, kind="ExternalInput"),
        "bq": nc.dram_tensor("bq", [P, HPC], F32, kind="ExternalInput"),
        "bk": nc.dram_tensor("bk", [P, HPC], F32, kind="ExternalInput"),
        "bvb": nc.dram_tensor("bvb", [P, CV], F32, kind="ExternalInput"),
        "o": nc.dram_tensor("o", [N, CV], F16, kind="ExternalOutput"),
    }
    with tile.TileContext(nc) as tc, ExitStack() as ctx:
        consts = ctx.enter_context(tc.tile_pool(name="consts", bufs=1))
        qk = ctx.enter_context(tc.tile_pool(name="qk", bufs=1))
        vaugp = ctx.enter_context(tc.tile_pool(name="vaugp", bufs=1))
        ep = ctx.enter_context(tc.tile_pool(name="ep", bufs=16))
        up = ctx.enter_context(tc.tile_pool(name="up", bufs=1))
        outp = ctx.enter_context(tc.tile_pool(name="outp", bufs=1))
        smallp = ctx.enter_context(tc.tile_pool(name="smallp", bufs=6))
        polyp = ctx.enter_context(tc.tile_pool(name="polyp", bufs=2))
        ps_s = ctx.enter_context(
            tc.tile_pool(name="ps_s", bufs=2, space="PSUM"))
        ps_w = ctx.enter_context(
            tc.tile_pool(name="ps_w", bufs=4, space="PSUM"))

        # ---- persistent tiles (addresses fixed across reps)
        xt_sb = [consts.tile([P, N], BF16, tag=f"xt{dc}", name=f"xt{dc}")
                 for dc in range(DC)]
        wq_sb = [consts.tile([P, CQ], BF16, tag=f"wq{dc}",
                             name=f"wq{dc}") for dc in range(DC)]
        wk_sb = [consts.tile([P, CQ], BF16, tag=f"wk{dc}",
                             name=f"wk{dc}") for dc in range(DC)]
        wv_all = consts.tile([P, DC * CV], BF16, tag="wv", name="wv")
        bq_sb = consts.tile([P, HPC], F32, tag="bq", name="bq")
        bk_sb = consts.tile([P, HPC], F32, tag="bk", name="bk")
        bvb_sb = consts.tile([P, CV], F32, tag="bvb", name="bvb")
        ident = consts.tile([P, P], F16, tag="ident", name="ident")
        ones_col = consts.tile([P, 1], BF16, tag="ones", name="ones")
        qt_t = [qk.tile([P, N], BF16, tag=f"qt{h}", name=f"qt{h}")
                for h in range(HPC)]
        kt_t = [qk.tile([P, N], BF16, tag=f"kt{h}", name=f"kt{h}")
                for h in range(HPC)]
        vaug = [vaugp.tile([P, HPC * AUG], BF16, tag=f"vaug{nt}",
                           name=f"vaug{nt}") for nt in range(NT)]
        u_sb = [[up.tile([AUG, N], F16, tag=f"u{h}_{hf}",
                         name=f"u{h}_{hf}") for hf in range(2)]
                for h in range(HPC)]
        corr_sb = [smallp.tile([AUG, 1], F32, tag=f"corr{h}", bufs=1,
                               name=f"corr{h}") for h in range(HPC)]
        ostage = outp.tile([P, NT * CV], F16, tag="ost", name="ost")
        # es tiles pre-allocated so wrapped chains can reference them
        es = {}
        for h in range(HPC):
            for kt in range(NT):
                es[(h, kt)] = ep.tile([P, 2 * N], BF16, tag="e",
                                      name=f"e{h}_{kt}")

        def input_dmas():
            for dc in range(DC):
                nc.sync.dma_start(xt_sb[dc][:],
                                  d["xt"][dc * P:(dc + 1) * P, :])
                nc.sync.dma_start(wq_sb[dc][:],
                                  d["wq"][dc * P:(dc + 1) * P, :])
                nc.sync.dma_start(wk_sb[dc][:],
                                  d["wk"][dc * P:(dc + 1) * P, :])
            nc.sync.dma_start(bq_sb[:], d["bq"][:])
            nc.sync.dma_start(bk_sb[:], d["bk"][:])
            nc.sync.dma_start(
                wv_all[:].rearrange("p (c n) -> p c n", c=DC),
                d["wv"].rearrange("(c p) n -> p c n", p=P))
            nc.sync.dma_start(bvb_sb[:], d["bvb"][:])

        def projqk(h, w_all, b_sb, dest, pfx):
            # one weight load per (h, dc), both qc matmuls reuse it
            ps = [ps_w.tile([P, QW], F32, tag="w",
                            name=f"ps_{pfx}{h}_{qc}") for qc in range(2)]
            for dc in range(DC):
                for qc in range(2):
                    nc.tensor.matmul(
                        ps[qc][:],
                        w_all[dc][:, h * P:(h + 1) * P],
                        xt_sb[dc][:, qc * QW:(qc + 1) * QW],
                        start=(dc == 0), stop=(dc == DC - 1))
            for qc in range(2):
                nc.vector.tensor_scalar_add(
                    dest[:, qc * QW:(qc + 1) * QW], ps[qc][:],
                    b_sb[:, h:h + 1])

        def vproj_chunk(nt):
            ps = ps_w.tile([P, QW], F32, tag="w", name=f"ps_v{nt}")
            psv = ps[:, 0:CV]
            for dc in range(DC):
                nc.tensor.matmul(
                    psv,
                    xt_sb[dc][:, nt * P:(nt + 1) * P],
                    wv_all[:, dc * CV:(dc + 1) * CV],
                    start=(dc == 0), stop=(dc == DC - 1))
            t1v = vaug[nt][:].rearrange("p (h a) -> p h a", a=AUG)
            nc.vector.tensor_add(
                t1v[:, :, 0:DH],
                psv.rearrange("p (h a) -> p h a", a=DH),
                bvb_sb[:].rearrange("p (h a) -> p h a", a=DH))

        def corr_mms(h):
            # corr[m] = sum_{kt in OFF_KTS} colsum(vaug_kt[:, h, m])
            cps = ps_w.tile([P, QW], F32, tag="w", name=f"cps{h}")
            for j, kt in enumerate(OFF_KTS):
                nc.tensor.matmul(
                    cps[0:AUG, 0:1],
                    vaug[kt][:, h * AUG:(h + 1) * AUG],
                    ones_col[:],
                    start=(j == 0), stop=(j == len(OFF_KTS) - 1))
            nc.vector.tensor_copy(corr_sb[h][:, 0:1], cps[0:AUG, 0:1])

        def chain_phase(h, half, qc):
            cps = ps_w.tile([AUG, QW], F32, tag="w",
                            name=f"pv{h}_{half}_{qc}")
            for kt in range(NT):
                nc.tensor.matmul(
                    cps[:],
                    vaug[kt][:, h * AUG:(h + 1) * AUG],
                    es[(h, kt)][:, half * N + qc * QW:
                                half * N + (qc + 1) * QW],
                    start=(kt == 0), stop=(kt == NT - 1))
            if OFF_KTS:
                nc.vector.tensor_scalar_add(
                    u_sb[h][half][0:AUG, qc * QW:(qc + 1) * QW], cps[:],
                    corr_sb[h][:, 0:1])
            else:
                nc.vector.tensor_copy(
                    u_sb[h][half][0:AUG, qc * QW:(qc + 1) * QW], cps[:])

        def tr_finish(h, qt_i, fire_dma):
            tr = ps_w.tile([P, 2 * AUG], F16, tag="w", name=f"tr{h}_{qt_i}")
            for hf in range(2):
                nc.tensor.transpose(
                    tr[:, hf * AUG:(hf + 1) * AUG],
                    u_sb[h][hf][0:AUG, qt_i * P:(qt_i + 1) * P],
                    ident[0:AUG, 0:AUG])
            # rr cols: 0 = 1/r1, 3 = 1/(-r2/LAM) = -LAM/r2
            trv = tr[:].rearrange("p (c a) -> p c a", a=AUG)
            rr = smallp.tile([P, 4], F32, tag="rr", name=f"rr{h}_{qt_i}")
            nc.vector.reciprocal(
                rr[:].rearrange("p (c a) -> p c a", a=2),
                trv[:, :, DH:DH + 2])
            rtmp = smallp.tile([P, DH], F16, tag="rtmp",
                               name=f"rt{h}_{qt_i}")
            nc.vector.tensor_scalar_mul(
                rtmp[:], tr[:, 0:DH], rr[:, 0:1])
            nc.vector.scalar_tensor_tensor(
                ostage[:, qt_i * CV + h * DH:qt_i * CV + (h + 1) * DH],
                tr[:, AUG:AUG + DH],
                rr[:, 3:4],
                rtmp[:],
                op0=mybir.AluOpType.mult,
                op1=mybir.AluOpType.add)
            if fire_dma:
                nc.sync.dma_start(
                    d["o"][qt_i * P:(qt_i + 1) * P, :],
                    ostage[:, qt_i * CV:(qt_i + 1) * CV])

        def poly_half(e, psh_half, h, kt, half):
            # w = s*(C1 + s*(C2 + s*C3)), s = SCALE*z via folded coeffs
            zb = polyp.tile([P, N], BF16, tag="zb",
                            name=f"zb{h}_{kt}_{half}")
            nc.vector.tensor_copy(zb[:], psh_half[:])
            t1 = polyp.tile([P, N], BF16, tag="t1",
                            name=f"t1{h}_{kt}_{half}")
            nc.vector.tensor_scalar(
                t1[:], zb[:], PC3, PC2,
                op0=mybir.AluOpType.mult, op1=mybir.AluOpType.add)
            t2 = polyp.tile([P, N], BF16, tag="t2",
                            name=f"t2{h}_{kt}_{half}")
            nc.vector.tensor_mul(t2[:], t1[:], zb[:])
            t3 = polyp.tile([P, N], BF16, tag="t3",
                            name=f"t3{h}_{kt}_{half}")
            nc.vector.tensor_scalar_add(t3[:], t2[:], PC1)
            nc.vector.tensor_mul(
                e[:, half * N:(half + 1) * N], t3[:], zb[:])

        def head_items(h, fire_dma):
            """(chains, finishes) fill items for head h. corr precedes
            the chains that consume it."""
            chains = [(("corr", h), 150, lambda h=h: corr_mms(h))]
            for qc in range(2):
                for half in range(2):
                    chains.append(
                        (("ch", h), 2100,
                         lambda h=h, half=half, qc=qc:
                         chain_phase(h, half, qc)))
            trs = [(("tr", h), 350,
                    lambda h=h, qt_i=qt_i: tr_finish(h, qt_i, fire_dma))
                   for qt_i in range(2 * HPC)]
            return chains, trs

        def score_steps(h, pump):
            qt, kt_ = qt_t[h], kt_t[h]
            for kt in range(NT):
                psh = [ps_s.tile([P, N], F32, tag="s",
                                 name=f"s{h}_{kt}_{hf}")
                       for hf in range(2)]
                for qc in range(2):
                    for half in range(2):
                        nc.tensor.matmul(
                            psh[half][:, qc * QW:(qc + 1) * QW],
                            kt_[half * DH:(half + 1) * DH,
                                kt * P:(kt + 1) * P],
                            qt[half * DH:(half + 1) * DH,
                               qc * QW:(qc + 1) * QW],
                            start=True, stop=True)
                e = es[(h, kt)]
                for half in range(2):
                    if (kt, half) in OFF_HALVES:
                        poly_half(e, psh[half][:], h, kt, half)
                    else:
                        nc.scalar.activation(
                            e[:, half * N:(half + 1) * N], psh[half][:],
                            mybir.ActivationFunctionType.Exp, scale=SCALE)
                pump(PUMP_NS)

        def body():
            input_dmas()
            # hi: deadline work (prev head's chains + next head's proj),
            # lo: latency-tolerant backlog (finishes, vproj, corr).
            hi = []
            lo = []

            def pump(budget):
                spent = 0
                while spent < budget and (hi or lo):
                    q = hi if hi else lo
                    _, cost, fn = q.pop(0)
                    fn()
                    spent += cost

            def mkproj(h, w_all, b_sb, dest, pfx):
                return (("proj", h), 2150,
                        lambda: projqk(h, w_all, b_sb, dest, pfx))

            def queue_head(h, hnext, fire_dma):
                # chains of head h (deadline: before exp(h+2) steps)
                chains, trs = head_items(h, fire_dma)
                hi.extend(chains[0:3])
                if hnext is not None:
                    hi.append(mkproj(hnext, wq_sb, bq_sb, qt_t[hnext], "q"))
                hi.extend(chains[3:5])
                if hnext is not None:
                    hi.append(mkproj(hnext, wk_sb, bk_sb, kt_t[hnext], "k"))
                lo.extend(trs)

            # wrapped: previous rep's h3 chains + h1 proj
            queue_head(HPC - 1, 1, fire_dma=False)
            for nt in range(NT):
                lo.append((("v",), 500, lambda nt=nt: vproj_chunk(nt)))

            for h in range(HPC):
                score_steps(h, pump)
                if h < HPC - 1:
                    # h's chains + proj for h+2 (h0's chains pair with
                    # proj h2; h1's with h3; h2's with next-rep h0)
                    hnext = h + 2 if h + 2 < HPC else 0
                    queue_head(h, hnext, fire_dma=(h == HPC - 2))

            while hi or lo:
                q = hi if hi else lo
                _, _, fn = q.pop(0)
                fn()

        def prologue():
            make_identity(nc, ident[:])
            nc.vector.memset(ones_col[:], 1.0)
            # pin the exp table set so the For_i body needs no reload
            scr = smallp.tile([P, 2], F32, tag="scr", bufs=1, name="scr")
            nc.scalar.activation(scr[:, 0:1], ones_col[:],
                                 mybir.ActivationFunctionType.Exp)
            for nt in range(NT):
                t1v = vaug[nt][:].rearrange("p (h a) -> p h a", a=AUG)
                nc.vector.memset(t1v[:, :, 0:DH], 0.0)
                nc.vector.memset(t1v[:, :, DH:DH + 1], 1.0)
                nc.vector.memset(t1v[:, :, DH + 1:AUG], -1.0 / LAM)
            # rep 0 of the pipelined body reads these before writing them
            # (1.0 keeps the garbage denominators finite)
            for kt in range(NT):
                nc.vector.memset(es[(HPC - 1, kt)][:], 1.0)
            for h in range(HPC):
                nc.vector.memset(corr_sb[h][:], 0.0)
            input_dmas()
            projqk(0, wq_sb, bq_sb, qt_t[0], "q")
            projqk(0, wk_sb, bk_sb, kt_t[0], "k")
            # vaug valid from rep 0 so body chains/corr can read it at
            # any point (per-rep vproj rewrites identical values)
            for nt in range(NT):
                vproj_chunk(nt)

        def epilogue():
            chains, trs = head_items(HPC - 1, fire_dma=True)
            for _, _, fn in chains + trs:
                fn()

        with nc.allow_low_precision(
                reason="fp16 u/denominators: |u|<=~1100, fp16 rel 5e-4"):
            prologue()
            if reps == 1:
                body()
            else:
                tc.prologue_barrier()
                with tc.For_i(0, reps, 1, staggered_reset=True,
                              hint_engines=(mybir.EngineType.PE,
                                            mybir.EngineType.DVE)):
                    body()
                tc.epilogue_barrier()
            epilogue()

    nc.compile()
    return nc


_NC_CACHE = {}


def get_nc(reps=1):
    if reps not in _NC_CACHE:
        _NC_CACHE[reps] = build_nc(reps)
    return _NC_CACHE[reps]


def shard_inputs(inputs):
    import ml_dtypes
    bf = np.dtype(ml_dtypes.bfloat16)
    x = np.asarray(inputs["x"], dtype=np.float32)
    Wq = np.asarray(inputs["Wq"], dtype=np.float32)
    bq = np.asarray(inputs["bq"], dtype=np.float32)
    Wk = np.asarray(inputs["Wk"], dtype=np.float32)
    bk = np.asarray(inputs["bk"], dtype=np.float32)
    Wv = np.asarray(inputs["Wv"], dtype=np.float32)
    bv = np.asarray(inputs["bv"], dtype=np.float32)
    in_maps = []
    for c in range(NCORES):
        b = c // 2
        h0 = (c % 2) * HPC
        cq0 = h0 * 2 * DH
        cv0 = h0 * DH
        in_maps.append({
            "xt": np.ascontiguousarray(x[b].T).astype(bf),
            "wq": np.ascontiguousarray(Wq[:, cq0:cq0 + CQ]).astype(bf),
            "wk": np.ascontiguousarray(Wk[:, cq0:cq0 + CQ]).astype(bf),
            "wv": np.ascontiguousarray(Wv[:, cv0:cv0 + CV]).astype(bf),
            "bq": np.ascontiguousarray(bq[cq0:cq0 + CQ].reshape(HPC, P).T),
            "bk": np.ascontiguousarray(bk[cq0:cq0 + CQ].reshape(HPC, P).T),
            "bvb": np.ascontiguousarray(
                np.broadcast_to(bv[cv0:cv0 + CV], (P, CV))),
        })
    return in_maps


def assemble_output(results):
    out = np.empty((B, N, D), dtype=np.float32)
    for c in range(NCORES):
        b = c // 2
        g = c % 2
        out[b, :, g * CV:(g + 1) * CV] = results[c]["o"].astype(np.float32)
    return out


def kernel(**inputs):
    nc = get_nc(1)
    in_maps = shard_inputs(inputs)
    res = bass_utils.run_bass_kernel_spmd(
        nc, in_maps, core_ids=list(range(NCORES)))
    return assemble_output(res.results)
